# revision 1
# baseline (speedup 1.0000x reference)
"""CrossAttentionBlock TRN2 kernel (8 NeuronCores), v2.

Sharding: core (b, g) = batch b in 0..3, head-group g in 0..1 (8 heads each).
Per core: Q/K/V projections for its head-group (fp16 matmuls), transposed
attention (softmax without max-subtraction; mask folded into the fused exp as
a per-partition bias; denominators via all-ones matmul + fast approx
reciprocal), partial output projection (fp16 partials), fp16 pairwise
ReduceScatter over (2b, 2b+1) at 256-row granularity, then fused residual +
LayerNorm on the owned rows, software-pipelined one chunk behind compute.

v2 over v1: all weights SBUF-resident (loaded once via single-descriptor
DMAs from host-prepacked [128, k, n] layouts), fp16 collective (half the
bytes), finer RS granularity + delayed LN to hide the collective tail,
reciprocal_approx_fast for softmax denominators, LN spread across
vector/scalar/gpsimd, and DMA issues kept off the busy queues.
"""

import os
import sys

sys.path.insert(0, "/opt/trn_rl_repo")

import numpy as np
from contextlib import ExitStack

import concourse.bass as bass
from concourse import bacc
import concourse.mybir as mybir
import concourse.tile as tile

F32 = mybir.dt.float32
F16 = mybir.dt.float16
AF = mybir.ActivationFunctionType
ALU = mybir.AluOpType

B, S, A, H, NH, DH = 4, 2048, 512, 2048, 16, 128
G = 2            # head groups (cores per batch)
HG = H // G      # 1024 channels per group
NHG = NH // G    # 8 heads per group
SCW = 512        # s-chunk width
NSC = S // SCW   # 4 chunks
HK = H // 128    # 16 contraction tiles
AC = A // 128    # 4 audio 128-blocks
OC4 = H // 512   # 4 output-channel 512-chunks
RPC = SCW // G   # 256 own rows per chunk
EPS = 1e-5
SM_SCALE = float(1.0 / np.sqrt(DH))

_CACHE = {}


def _build():
    nc = bacc.Bacc("TRN2", target_bir_lowering=False, debug=False, num_devices=8)

    xt16 = nc.dram_tensor("xt16", [128, HK, S], F16, kind="ExternalInput").ap()
    aut16 = nc.dram_tensor("aut16", [128, HK, A], F16, kind="ExternalInput").ap()
    wq16 = nc.dram_tensor("wq16", [128, HK, HG], F16, kind="ExternalInput").ap()
    wk16 = nc.dram_tensor("wk16", [128, HK, HG], F16, kind="ExternalInput").ap()
    wv16 = nc.dram_tensor("wv16", [128, HK, HG], F16, kind="ExternalInput").ap()
    wo16 = nc.dram_tensor("wo16", [128, NHG, H], F16, kind="ExternalInput").ap()
    ones16 = nc.dram_tensor("ones16", [128, 128], F16, kind="ExternalInput").ap()
    bqT = nc.dram_tensor("bqT", [128, NHG], F32, kind="ExternalInput").ap()
    bkT = nc.dram_tensor("bkT", [128, NHG], F32, kind="ExternalInput").ap()
    maskT = nc.dram_tensor("maskT", [128, AC], F32, kind="ExternalInput").ap()
    resid16 = nc.dram_tensor(
        "resid16", [S // G, OC4, 512], F16, kind="ExternalInput"
    ).ap()
    gamma16 = nc.dram_tensor("gamma16", [128, OC4, 512], F16,
                             kind="ExternalInput").ap()
    beta16 = nc.dram_tensor("beta16", [128, OC4, 512], F16,
                            kind="ExternalInput").ap()

    po_in16 = nc.dram_tensor("po_in16", [S, OC4, 512], F16)
    po_out16 = nc.dram_tensor("po_out16", [S // G, OC4, 512], F16)
    y16 = nc.dram_tensor("y16", [S // G, OC4, 512], F16, kind="ExternalOutput").ap()

    groups = [[0, 1], [2, 3], [4, 5], [6, 7]]

    with tile.TileContext(nc) as tc:
        with ExitStack() as ctx:
            # ---------------- resident tiles + constants ----------------
            cpool = ctx.enter_context(tc.tile_pool(name="consts", bufs=1))
            wpool = ctx.enter_context(tc.tile_pool(name="weights", bufs=1))

            wq_sb = wpool.tile([128, HK, HG], F16)
            wo_sb = wpool.tile([128, NHG, H], F16)
            KT = wpool.tile([128, NHG, A], F16)      # [dh, head, a]
            V = wpool.tile([128, AC, HG], F16)       # [a_in_blk, a_blk, vc]

            ones_sb = cpool.tile([128, 128], F16)
            bq_sb = cpool.tile([128, NHG], F32)
            bk_sb = cpool.tile([128, NHG], F32)
            mask_sb = cpool.tile([128, AC], F32)
            eps_sb = cpool.tile([128, 1], F32)
            gamma_sb = cpool.tile([128, OC4, 512], F16)
            beta_sb = cpool.tile([128, OC4, 512], F16)

            # xt pool coexists with phase A (chunk-0 prefetch); the rest of
            # the main-loop pools are created after phase A frees its
            # transients so they can reuse that SBUF range.
            xpool = ctx.enter_context(tc.tile_pool(name="xts", bufs=2))

            # ---------------- initial loads (spread across queues) ------
            # sync: aut halves then xt chunk 0 (kproj gate first).
            # gpsimd: wk halves then wv halves.
            # vector: wq, wo (needed later).
            # scalar: small constants.
            with ExitStack() as actx:
                apool = actx.enter_context(tc.tile_pool(name="phA", bufs=1))
                aut_sb = apool.tile([128, HK, A], F16)
                wk_sb = apool.tile([128, HK, HG], F16)
                wv_sb = apool.tile([128, HK, HG], F16)

                nc.sync.dma_start(aut_sb[:, 0:HK // 2, :],
                                  aut16[:, 0:HK // 2, :])
                nc.gpsimd.dma_start(wk_sb[:, 0:HK // 2, :],
                                    wk16[:, 0:HK // 2, :])
                nc.sync.dma_start(aut_sb[:, HK // 2:HK, :],
                                  aut16[:, HK // 2:HK, :])
                nc.gpsimd.dma_start(wk_sb[:, HK // 2:HK, :],
                                    wk16[:, HK // 2:HK, :])
                nc.gpsimd.dma_start(wv_sb[:, 0:HK // 2, :],
                                    wv16[:, 0:HK // 2, :])
                nc.gpsimd.dma_start(wv_sb[:, HK // 2:HK, :],
                                    wv16[:, HK // 2:HK, :])
                nc.sync.dma_start(wq_sb[:], wq16)
                nc.scalar.dma_start(wo_sb[:], wo16)
                nc.scalar.dma_start(ones_sb[:], ones16)
                nc.scalar.dma_start(bq_sb[:], bqT)
                nc.scalar.dma_start(bk_sb[:], bkT)
                nc.scalar.dma_start(mask_sb[:], maskT)
                nc.scalar.dma_start(gamma_sb[:], gamma16)
                nc.scalar.dma_start(beta_sb[:], beta16)
                nc.vector.memset(eps_sb[:], EPS)

                xt_tiles = {}
                xt_tiles[0] = xpool.tile([128, HK, SCW], F16, tag="xt", bufs=2,
                                         name="xt0")
                nc.sync.dma_start(xt_tiles[0][:], xt16[:, :, 0:SCW])

                # ---------------- phase A: K^T and V ----------------
                apsum = actx.enter_context(
                    tc.tile_pool(name="phAp", bufs=8, space="PSUM")
                )
                with nc.named_scope("kproj"):
                    psk = []
                    for m in range(NHG):
                        pk = apsum.tile([128, A], F32, tag="pk", bufs=8)
                        psk.append(pk)
                    for hk in range(HK):
                        for m in range(NHG):
                            nc.tensor.matmul(
                                psk[m][:],
                                wk_sb[:, hk, m * 128:(m + 1) * 128],
                                aut_sb[:, hk, :],
                                start=(hk == 0),
                                stop=(hk == HK - 1),
                            )
                    for m in range(NHG):
                        nc.scalar.activation(
                            KT[:, m, :], psk[m][:], AF.Identity,
                            bias=bk_sb[:, m:m + 1],
                        )
                with nc.named_scope("vproj"):
                    psv = []
                    for i in range(8):
                        pv = apsum.tile([128, 512], F32, tag="pk", bufs=8)
                        psv.append(pv)
                    for hk in range(HK):
                        for ac in range(AC):
                            for n in range(2):
                                nc.tensor.matmul(
                                    psv[ac * 2 + n][:],
                                    aut_sb[:, hk, ac * 128:(ac + 1) * 128],
                                    wv_sb[:, hk, n * 512:(n + 1) * 512],
                                    start=(hk == 0),
                                    stop=(hk == HK - 1),
                                )
                    for ac in range(AC):
                        for n in range(2):
                            nc.scalar.copy(
                                V[:, ac, n * 512:(n + 1) * 512],
                                psv[ac * 2 + n][:],
                            )

            # ---------------- main chunk loop ----------------
            mpsum = ctx.enter_context(
                tc.tile_pool(name="mps", bufs=1, space="PSUM")
            )
            qpool = ctx.enter_context(tc.tile_pool(name="qts", bufs=1))
            epool = ctx.enter_context(tc.tile_pool(name="eps", bufs=8))
            rpool = ctx.enter_context(tc.tile_pool(name="rcs", bufs=2))
            ctpool = ctx.enter_context(tc.tile_pool(name="cts", bufs=1))
            opool = ctx.enter_context(tc.tile_pool(name="ots", bufs=2))
            lpool = ctx.enter_context(tc.tile_pool(name="lns", bufs=2))
            spool = ctx.enter_context(tc.tile_pool(name="lsc", bufs=4))

            def ln_tile(cp, h):
                r0 = cp * RPC + h * 128
                po_sb = lpool.tile([128, OC4, 512], F16, tag="po16", bufs=2,
                                   name=f"po16_{cp}{h}")
                nc.gpsimd.dma_start(po_sb[:], po_out16.ap()[r0:r0 + 128, :, :])
                rt = lpool.tile([128, OC4, 512], F16, tag="resid", bufs=2,
                                name=f"res{cp}{h}")
                nc.gpsimd.dma_start(rt[:], resid16[r0:r0 + 128, :, :])
                x_t = lpool.tile([128, OC4, 512], F32, tag="x", bufs=2,
                                 name=f"x{cp}{h}")
                nc.vector.tensor_tensor(x_t[:], po_sb[:], rt[:], ALU.add)
                xsum = spool.tile([128, 1], F32, tag="xsum", bufs=4)
                nc.vector.tensor_reduce(
                    xsum[:], x_t[:], mybir.AxisListType.XY, ALU.add
                )
                nmu = spool.tile([128, 1], F32, tag="nmu", bufs=4)
                nc.vector.tensor_scalar(nmu[:], xsum[:], -1.0 / H, None,
                                        ALU.mult)
                ssq = spool.tile([128, 1], F32, tag="ssq", bufs=4)
                xn = lpool.tile([128, OC4, 512], F16, tag="xn", bufs=2,
                                name=f"xn{cp}{h}")
                nc.scalar.activation(
                    xn[:], x_t[:], AF.Square, bias=nmu[:], accum_out=ssq[:],
                )
                std = spool.tile([128, 1], F32, tag="std", bufs=4)
                nc.scalar.activation(
                    std[:], ssq[:], AF.Sqrt, scale=1.0 / H, bias=eps_sb[:],
                )
                rstd = spool.tile([128, 1], F32, tag="rstd", bufs=4)
                nc.vector.reciprocal(rstd[:], std[:])
                nmr = spool.tile([128, 1], F32, tag="nmr", bufs=4)
                nc.vector.tensor_scalar(nmr[:], nmu[:], rstd[:], None,
                                        ALU.mult)
                nc.scalar.activation(
                    xn[:], x_t[:], AF.Identity, scale=rstd[:], bias=nmr[:],
                )
                nc.gpsimd.tensor_mul(xn[:], xn[:], gamma_sb[:])
                nc.gpsimd.tensor_add(xn[:], xn[:], beta_sb[:])
                nc.gpsimd.dma_start(y16[r0:r0 + 128, :, :], xn[:])

            for sc in range(NSC):
                s0 = sc * SCW
                if sc + 1 < NSC:
                    xt_tiles[sc + 1] = xpool.tile(
                        [128, HK, SCW], F16, tag="xt", bufs=2,
                        name=f"xt{sc + 1}",
                    )
                    nc.sync.dma_start(
                        xt_tiles[sc + 1][:],
                        xt16[:, :, s0 + SCW:s0 + 2 * SCW],
                    )
                xt_c = xt_tiles.pop(sc)

                # ---- Q^T projection ----
                qt = qpool.tile([128, NHG, SCW], F16, tag="qt", bufs=1,
                                name=f"qt{sc}")
                with nc.named_scope("qproj"):
                    for m in range(NHG):
                        pq = mpsum.tile([128, SCW], F32, tag="pq", bufs=2)
                        for hk in range(HK):
                            nc.tensor.matmul(
                                pq[:],
                                wq_sb[:, hk, m * 128:(m + 1) * 128],
                                xt_c[:, hk, :],
                                start=(hk == 0),
                                stop=(hk == HK - 1),
                            )
                        nc.vector.tensor_scalar_add(
                            qt[:, m, :], pq[:], bq_sb[:, m:m + 1],
                        )

                # ---- attention (softmax-pipelined over heads) ----
                ct = ctpool.tile([128, NHG, SCW], F16, tag="ct", bufs=1,
                                 name=f"ct{sc}")
                with nc.named_scope("attn"):
                    eps_by_head = {}

                    def scores(h):
                        eps_h = []
                        for ac in range(AC):
                            pp = mpsum.tile([128, SCW], F32, tag="pp", bufs=2)
                            nc.tensor.matmul(
                                pp[:],
                                KT[:, h, ac * 128:(ac + 1) * 128],
                                qt[:, h, :],
                                start=True, stop=True,
                            )
                            ep = epool.tile([128, SCW], F16, tag="ep", bufs=8)
                            nc.scalar.activation(
                                ep[:], pp[:], AF.Exp,
                                bias=mask_sb[:, ac:ac + 1], scale=SM_SCALE,
                            )
                            eps_h.append(ep)
                        eps_by_head[h] = eps_h

                    def finish(h):
                        eps_h = eps_by_head.pop(h)
                        ps = mpsum.tile([128, SCW], F32, tag="ps", bufs=1)
                        for ac in range(AC):
                            nc.tensor.matmul(
                                ps[:], ones_sb[:], eps_h[ac][:],
                                start=(ac == 0), stop=(ac == AC - 1),
                            )
                        pc = mpsum.tile([128, SCW], F32, tag="pc", bufs=1)
                        for ac in range(AC):
                            nc.tensor.matmul(
                                pc[:],
                                V[:, ac, h * 128:(h + 1) * 128],
                                eps_h[ac][:],
                                start=(ac == 0), stop=(ac == AC - 1),
                            )
                        rc = rpool.tile([128, SCW], F32, tag="rc", bufs=2)
                        nc.vector.reciprocal_approx_fast(rc[:], ps[:])
                        nc.vector.tensor_tensor(
                            ct[:, h, :], pc[:], rc[:], ALU.mult,
                        )

                    scores(0)
                    scores(1)
                    for h in range(NHG):
                        finish(h)
                        if h + 2 < NHG:
                            scores(h + 2)

                # ---- partial out-projection (mq-major) + RS halves ----
                with nc.named_scope("outproj"):
                    for mq in range(4):
                        ot = opool.tile([128, OC4, 512], F16, tag="ot",
                                        bufs=2)
                        for n in range(OC4):
                            po = mpsum.tile([128, 512], F32, tag="po", bufs=2)
                            for c in range(NHG):
                                nc.tensor.matmul(
                                    po[:],
                                    ct[:, c, mq * 128:(mq + 1) * 128],
                                    wo_sb[:, c, n * 512:(n + 1) * 512],
                                    start=(c == 0), stop=(c == NHG - 1),
                                )
                            nc.scalar.copy(ot[:, n, :], po[:])
                        nc.sync.dma_start(
                            po_in16.ap()[s0 + mq * 128:s0 + (mq + 1) * 128,
                                         :, :],
                            ot[:],
                        )
                        if mq % 2 == 1:
                            h = mq // 2
                            with nc.named_scope("rs"):
                                nc.gpsimd.collective_compute(
                                    "ReduceScatter",
                                    ALU.add,
                                    replica_groups=groups,
                                    ins=[po_in16.ap()[
                                        s0 + h * 256:s0 + (h + 1) * 256,
                                        :, :].opt()],
                                    outs=[po_out16.ap()[
                                        sc * RPC + h * 128:
                                        sc * RPC + (h + 1) * 128,
                                        :, :].opt()],
                                )

                # ---- delayed LN for previous chunk ----
                if sc > 0:
                    with nc.named_scope("ln"):
                        for h in range(G):
                            ln_tile(sc - 1, h)

            with nc.named_scope("ln"):
                for h in range(G):
                    ln_tile(NSC - 1, h)

    nc.compile()
    return nc


def _get_nc():
    if "nc" not in _CACHE:
        _CACHE["nc"] = _build()
    return _CACHE["nc"]


def _own_rows(g):
    return np.array(
        [c * SCW + h * 256 + g * 128 + i
         for c in range(NSC) for h in range(G) for i in range(128)]
    )


def _prep_in_maps(hidden_states, audio_tokens, attention_mask,
                  Wq, bq, Wk, bk, Wv, bv, Wo, bo, gamma, beta):
    f = np.float32
    h16 = np.float16
    hs = np.asarray(hidden_states, f)
    au = np.asarray(audio_tokens, f)
    am = np.asarray(attention_mask, f)
    Wq, bq = np.asarray(Wq, f), np.asarray(bq, f)
    Wk, bk = np.asarray(Wk, f), np.asarray(bk, f)
    Wv, bv = np.asarray(Wv, f), np.asarray(bv, f)
    Wo, bo = np.asarray(Wo, f), np.asarray(bo, f)
    gamma, beta = np.asarray(gamma, f), np.asarray(beta, f)

    bo_eff = bo + bv @ Wo  # fold the V bias through the output projection
    ones = np.ones((128, 128), h16)
    gamma_b = np.ascontiguousarray(
        np.broadcast_to(gamma, (128, H))).astype(h16).reshape(128, OC4, 512)
    beta_b = np.ascontiguousarray(
        np.broadcast_to(beta, (128, H))).astype(h16).reshape(128, OC4, 512)

    in_maps = []
    for b in range(B):
        xt = np.ascontiguousarray(
            hs[b].T.reshape(HK, 128, S).transpose(1, 0, 2)).astype(h16)
        autb = np.ascontiguousarray(
            au[b].T.reshape(HK, 128, A).transpose(1, 0, 2)).astype(h16)
        maskT = np.ascontiguousarray((am[b] * -10000.0).reshape(AC, 128).T)
        for g in range(G):
            sl = slice(g * HG, (g + 1) * HG)
            rows = _own_rows(g)
            in_maps.append({
                "xt16": xt,
                "aut16": autb,
                "wq16": np.ascontiguousarray(
                    Wq[:, sl].reshape(HK, 128, HG).transpose(1, 0, 2)
                ).astype(h16),
                "wk16": np.ascontiguousarray(
                    Wk[:, sl].reshape(HK, 128, HG).transpose(1, 0, 2)
                ).astype(h16),
                "wv16": np.ascontiguousarray(
                    Wv[:, sl].reshape(HK, 128, HG).transpose(1, 0, 2)
                ).astype(h16),
                "wo16": np.ascontiguousarray(
                    Wo[sl, :].reshape(NHG, 128, H).transpose(1, 0, 2)
                ).astype(h16),
                "ones16": ones,
                "bqT": np.ascontiguousarray(bq[sl].reshape(NHG, 128).T),
                "bkT": np.ascontiguousarray(bk[sl].reshape(NHG, 128).T),
                "maskT": maskT,
                "resid16": (hs[b][rows] + bo_eff[None, :]).astype(h16)
                .reshape(S // G, OC4, 512),
                "gamma16": gamma_b,
                "beta16": beta_b,
            })
    return in_maps


def run_sharded(in_maps, trace=False):
    from concourse.bass_utils import run_bass_kernel_spmd

    nc = _get_nc()
    return run_bass_kernel_spmd(
        nc, in_maps, core_ids=list(range(8)), trace=trace,
        trace_cores=[0] if trace else None,
    )


def kernel(**inputs) -> np.ndarray:
    in_maps = _prep_in_maps(**inputs)
    trace = bool(int(os.environ.get("BASS_KERNEL_TRACE", "0")))
    r = run_sharded(in_maps, trace=trace)
    _CACHE["last_result"] = r
    out = np.empty((B, S, H), np.float32)
    for b in range(B):
        for g in range(G):
            out[b][_own_rows(g)] = (
                r.results[b * G + g]["y16"].astype(np.float32)
                .reshape(S // G, H)
            )
    return out



# revision 5
# speedup vs baseline: 1.2655x; 1.2655x over previous
"""CrossAttentionBlock TRN2 kernel (8 NeuronCores), v3.

Sharding v3: core (b, g) = batch b in 0..3, sequence-half g in 0..1
(1024 rows each). Each core computes K/V projections for its 8 "own"
heads, exchanges them with its pair partner via a single early 2 MB
AllReduce (kvsum = KV_own + KV_peer; peer = kvsum - own, subtracted on
gpsimd), then runs the full 16-head attention + output projection +
residual/LayerNorm for its own rows entirely locally — no tail
collective, no partial-output HBM round-trip.

Head slots are ordered [own heads | peer heads] per core via host-side
weight packing, so the program is rank-uniform: own heads (slots 0-7)
never depend on the collective, and the AllReduce hides under the
~110 us Q projection. Tensor work: 1664 matmuls/core (same as v2) but
with no ReduceScatter tail, no mid-kernel RS stall, and a shorter
startup DMA ramp (wk/wv streamed at 2-hk granularity).
"""

import os
import sys

sys.path.insert(0, "/opt/trn_rl_repo")

import numpy as np
from contextlib import ExitStack

import concourse.bass as bass
from concourse import bacc
import concourse.mybir as mybir
import concourse.tile as tile

F32 = mybir.dt.float32
F16 = mybir.dt.float16
AF = mybir.ActivationFunctionType
ALU = mybir.AluOpType

B, S, A, H, NH, DH = 4, 2048, 512, 2048, 16, 128
G = 2            # sequence halves (cores per batch)
SL = S // G      # 1024 own rows per core
NHG = NH // G    # 8 own heads per core
HG = H // G      # 1024 channels per head-group
SCW = 512        # s-chunk width
NSC = SL // SCW  # 2 local chunks
HK = H // 128    # 16 contraction tiles
AC = A // 128    # 4 audio 128-blocks
OC4 = H // 512   # 4 output-channel 512-chunks
EPS = 1e-5
SM_SCALE = float(1.0 / np.sqrt(DH))
AGK = NHG * A        # 4096 flat cols of own K^T in the exchange buffer
AGV = AC * HG        # 4096 flat cols of own V

_CACHE = {}


def _build():
    nc = bacc.Bacc("TRN2", target_bir_lowering=False, debug=False, num_devices=8)

    xt16 = nc.dram_tensor("xt16", [128, HK, SL], F16, kind="ExternalInput").ap()
    aut16 = nc.dram_tensor("aut16", [128, HK, A], F16, kind="ExternalInput").ap()
    wq16 = nc.dram_tensor("wq16", [128, HK, H], F16, kind="ExternalInput").ap()
    wk16 = nc.dram_tensor("wk16", [128, HK, HG], F16, kind="ExternalInput").ap()
    wv16 = nc.dram_tensor("wv16", [128, HK, HG], F16, kind="ExternalInput").ap()
    wo16 = nc.dram_tensor("wo16", [128, NH, H], F16, kind="ExternalInput").ap()
    ones16 = nc.dram_tensor("ones16", [128, 128], F16, kind="ExternalInput").ap()
    bqT = nc.dram_tensor("bqT", [128, NH], F32, kind="ExternalInput").ap()
    bkT = nc.dram_tensor("bkT", [128, NHG], F32, kind="ExternalInput").ap()
    maskT = nc.dram_tensor("maskT", [128, AC], F32, kind="ExternalInput").ap()
    resid16 = nc.dram_tensor("resid16", [SL, OC4, 512], F16,
                             kind="ExternalInput").ap()
    gamma16 = nc.dram_tensor("gamma16", [128, OC4, 512], F16,
                             kind="ExternalInput").ap()
    beta16 = nc.dram_tensor("beta16", [128, OC4, 512], F16,
                            kind="ExternalInput").ap()

    ag_ink = nc.dram_tensor("ag_ink", [128, NHG, A], F16)
    ag_inv = nc.dram_tensor("ag_inv", [128, AC, HG], F16)
    ag_outk = nc.dram_tensor("ag_outk", [128, NHG, A], F16)
    ag_outv = nc.dram_tensor("ag_outv", [128, AC, HG], F16)
    y16 = nc.dram_tensor("y16", [SL, OC4, 512], F16, kind="ExternalOutput").ap()

    groups = [[0, 1], [2, 3], [4, 5], [6, 7]]

    with tile.TileContext(nc) as tc:
        with ExitStack() as ctx:
            # ---------------- persistent pools ----------------
            cpool = ctx.enter_context(tc.tile_pool(name="consts", bufs=1))
            wpool = ctx.enter_context(tc.tile_pool(name="weights", bufs=1))
            kvpool = ctx.enter_context(tc.tile_pool(name="kv", bufs=1))
            spool = ctx.enter_context(tc.tile_pool(name="sbig", bufs=1))

            KT = kvpool.tile([128, NH, A], F16)      # [dh, head-slot, a]
            V = kvpool.tile([128, AC, H], F16)       # [a_in_blk, a_blk, vc-slot]

            ones_sb = cpool.tile([128, 128], F16)
            bq_sb = cpool.tile([128, NH], F32)
            bk_sb = cpool.tile([128, NHG], F32)
            mask_sb = cpool.tile([128, AC], F32)
            eps_sb = cpool.tile([128, 1], F32)
            gamma_sb = cpool.tile([128, OC4, 512], F16)
            beta_sb = cpool.tile([128, OC4, 512], F16)

            # wq now; wo later reuses the same 64KB/partition slot.
            wq_sb = wpool.tile([128, HK, H], F16, tag="w", bufs=1, name="wq_sb")

            # xt0, xt1, qt0, qt1 then ct0 -> xt0's slot, ct1 -> xt1's.
            xt_t = {}
            for c in range(NSC):
                xt_t[c] = spool.tile([128, HK, SCW], F16, tag="sbig", bufs=4,
                                     name=f"xt{c}")

            # ---------------- initial loads ----------------
            nc.sync.dma_start(xt_t[0][:], xt16[:, :, 0:SCW])
            nc.sync.dma_start(wq_sb[:], wq16)
            nc.sync.dma_start(xt_t[1][:], xt16[:, :, SCW:2 * SCW])
            nc.scalar.dma_start(ones_sb[:], ones16)
            nc.scalar.dma_start(bq_sb[:], bqT)
            nc.scalar.dma_start(bk_sb[:], bkT)
            nc.scalar.dma_start(mask_sb[:], maskT)
            nc.scalar.dma_start(gamma_sb[:], gamma16)
            nc.scalar.dma_start(beta_sb[:], beta16)
            nc.vector.memset(eps_sb[:], EPS)

            # ---------------- phase A: own-head K^T and V ----------------
            with ExitStack() as actx:
                apool = actx.enter_context(tc.tile_pool(name="phA", bufs=1))
                apsum = actx.enter_context(
                    tc.tile_pool(name="phAp", bufs=8, space="PSUM")
                )
                aut_sb = apool.tile([128, HK, A], F16)
                nc.gpsimd.dma_start(aut_sb[:, 0:HK // 2, :],
                                    aut16[:, 0:HK // 2, :])
                nc.gpsimd.dma_start(aut_sb[:, HK // 2:HK, :],
                                    aut16[:, HK // 2:HK, :])

                with nc.named_scope("kproj"):
                    psk = []
                    for m in range(NHG):
                        pk = apsum.tile([128, A], F32, tag="pk", bufs=8)
                        psk.append(pk)
                    for i in range(HK // 2):
                        wkt = apool.tile([128, 2, HG], F16, tag="wkv", bufs=3,
                                         name=f"wk{i}")
                        nc.gpsimd.dma_start(wkt[:], wk16[:, 2 * i:2 * i + 2, :])
                        for j in range(2):
                            hk = 2 * i + j
                            for m in range(NHG):
                                nc.tensor.matmul(
                                    psk[m][:],
                                    wkt[:, j, m * 128:(m + 1) * 128],
                                    aut_sb[:, hk, :],
                                    start=(hk == 0),
                                    stop=(hk == HK - 1),
                                )
                    for m in range(NHG):
                        nc.scalar.activation(
                            KT[:, m, :], psk[m][:], AF.Identity,
                            bias=bk_sb[:, m:m + 1],
                        )

                with nc.named_scope("vproj"):
                    psv = []
                    for i in range(8):
                        pv = apsum.tile([128, 512], F32, tag="pk", bufs=8)
                        psv.append(pv)
                    for i in range(HK // 2):
                        wvt = apool.tile([128, 2, HG], F16, tag="wkv", bufs=3,
                                         name=f"wv{i}")
                        nc.gpsimd.dma_start(wvt[:], wv16[:, 2 * i:2 * i + 2, :])
                        for j in range(2):
                            hk = 2 * i + j
                            for ac in range(AC):
                                for n in range(2):
                                    nc.tensor.matmul(
                                        psv[ac * 2 + n][:],
                                        aut_sb[:, hk, ac * 128:(ac + 1) * 128],
                                        wvt[:, j, n * 512:(n + 1) * 512],
                                        start=(hk == 0),
                                        stop=(hk == HK - 1),
                                    )
                    for ac in range(AC):
                        for n in range(2):
                            nc.scalar.copy(
                                V[:, ac, n * 512:(n + 1) * 512],
                                psv[ac * 2 + n][:],
                            )

            # ---------------- K/V exchange: AllReduce + subtract ---------
            # kvsum = KV_own + KV_peer on both ranks (rank-uniform program);
            # peer = kvsum - own, computed in place on gpsimd so nothing
            # outside the gpsimd queue ever waits on the collective.
            with nc.named_scope("kvx"):
                nc.gpsimd.dma_start(ag_ink.ap()[:], KT[:, 0:NHG, :])
                nc.gpsimd.dma_start(ag_inv.ap()[:], V[:, :, 0:HG])
                nc.gpsimd.collective_compute(
                    "AllReduce",
                    ALU.add,
                    replica_groups=groups,
                    ins=[ag_ink.ap().opt()],
                    outs=[ag_outk.ap().opt()],
                )
                nc.gpsimd.collective_compute(
                    "AllReduce",
                    ALU.add,
                    replica_groups=groups,
                    ins=[ag_inv.ap().opt()],
                    outs=[ag_outv.ap().opt()],
                )
                nc.gpsimd.dma_start(KT[:, NHG:NH, :], ag_outk.ap()[:])
                nc.gpsimd.dma_start(V[:, :, HG:H], ag_outv.ap()[:])
                nc.gpsimd.tensor_sub(KT[:, NHG:NH, :], KT[:, NHG:NH, :],
                                     KT[:, 0:NHG, :])
                nc.gpsimd.tensor_sub(V[:, :, HG:H], V[:, :, HG:H],
                                     V[:, :, 0:HG])

            # ---------------- main pools ----------------
            mpsum = ctx.enter_context(
                tc.tile_pool(name="mps", bufs=1, space="PSUM")
            )
            epool = ctx.enter_context(tc.tile_pool(name="eps", bufs=8))
            rpool = ctx.enter_context(tc.tile_pool(name="rcs", bufs=2))
            lpool = ctx.enter_context(tc.tile_pool(name="lns", bufs=2))
            stpool = ctx.enter_context(tc.tile_pool(name="lsc", bufs=4))

            # ---------------- Q projection (both chunks) ----------------
            qt_t = {}
            with nc.named_scope("qproj"):
                for c in range(NSC):
                    qt_t[c] = spool.tile([128, NH, SCW], F16, tag="sbig",
                                         bufs=4, name=f"qt{c}")
                    for m in range(NH):
                        pq = mpsum.tile([128, SCW], F32, tag="pq", bufs=2)
                        for hk in range(HK):
                            nc.tensor.matmul(
                                pq[:],
                                wq_sb[:, hk, m * 128:(m + 1) * 128],
                                xt_t[c][:, hk, :],
                                start=(hk == 0),
                                stop=(hk == HK - 1),
                            )
                        nc.vector.tensor_scalar_add(
                            qt_t[c][:, m, :], pq[:], bq_sb[:, m:m + 1],
                        )

            # wo replaces wq once the Q projection has consumed it.
            wo_sb = wpool.tile([128, NH, H], F16, tag="w", bufs=1, name="wo_sb")
            nc.sync.dma_start(wo_sb[:], wo16)

            # ---------------- attention (both chunks, softmax-pipelined) --
            ct_t = {}
            for c in range(NSC):
                ct_t[c] = spool.tile([128, NH, SCW], F16, tag="sbig", bufs=4,
                                     name=f"ct{c}")
            with nc.named_scope("attn"):
                eps_by_head = {}

                def scores(c, h):
                    eps_h = []
                    for ac in range(AC):
                        pp = mpsum.tile([128, SCW], F32, tag="pp", bufs=2)
                        nc.tensor.matmul(
                            pp[:],
                            KT[:, h, ac * 128:(ac + 1) * 128],
                            qt_t[c][:, h, :],
                            start=True, stop=True,
                        )
                        ep = epool.tile([128, SCW], F16, tag="ep", bufs=8)
                        nc.scalar.activation(
                            ep[:], pp[:], AF.Exp,
                            bias=mask_sb[:, ac:ac + 1], scale=SM_SCALE,
                        )
                        eps_h.append(ep)
                    eps_by_head[(c, h)] = eps_h

                def finish(c, h):
                    eps_h = eps_by_head.pop((c, h))
                    ps = mpsum.tile([128, SCW], F32, tag="ps", bufs=1)
                    for ac in range(AC):
                        nc.tensor.matmul(
                            ps[:], ones_sb[:], eps_h[ac][:],
                            start=(ac == 0), stop=(ac == AC - 1),
                        )
                    pc = mpsum.tile([128, SCW], F32, tag="pc", bufs=1)
                    for ac in range(AC):
                        nc.tensor.matmul(
                            pc[:],
                            V[:, ac, h * 128:(h + 1) * 128],
                            eps_h[ac][:],
                            start=(ac == 0), stop=(ac == AC - 1),
                        )
                    rc = rpool.tile([128, SCW], F32, tag="rc", bufs=2)
                    nc.vector.reciprocal_approx_fast(rc[:], ps[:])
                    nc.vector.tensor_tensor(
                        ct_t[c][:, h, :], pc[:], rc[:], ALU.mult,
                    )

                hseq = [(c, h) for c in range(NSC) for h in range(NH)]
                scores(*hseq[0])
                scores(*hseq[1])
                for i, (c, h) in enumerate(hseq):
                    finish(c, h)
                    if i + 2 < len(hseq):
                        scores(*hseq[i + 2])

            # ---------------- out proj + residual + LayerNorm ------------
            def ln_tile(c, mq, x_t):
                """LN for 128 rows given x = out+resid already in x_t."""
                r0 = c * SCW + mq * 128
                xsum = stpool.tile([128, 1], F32, tag="xsum", bufs=4)
                nc.vector.tensor_reduce(
                    xsum[:], x_t[:], mybir.AxisListType.XY, ALU.add
                )
                nmu = stpool.tile([128, 1], F32, tag="nmu", bufs=4)
                nc.vector.tensor_scalar(nmu[:], xsum[:], -1.0 / H, None,
                                        ALU.mult)
                ssq = stpool.tile([128, 1], F32, tag="ssq", bufs=4)
                xn = lpool.tile([128, OC4, 512], F16, tag="xn", bufs=2,
                                name=f"xn{c}{mq}")
                nc.scalar.activation(
                    xn[:], x_t[:], AF.Square, bias=nmu[:], accum_out=ssq[:],
                )
                std = stpool.tile([128, 1], F32, tag="std", bufs=4)
                nc.scalar.activation(
                    std[:], ssq[:], AF.Sqrt, scale=1.0 / H, bias=eps_sb[:],
                )
                rstd = stpool.tile([128, 1], F32, tag="rstd", bufs=4)
                nc.vector.reciprocal(rstd[:], std[:])
                nmr = stpool.tile([128, 1], F32, tag="nmr", bufs=4)
                nc.vector.tensor_scalar(nmr[:], nmu[:], rstd[:], None,
                                        ALU.mult)
                nc.scalar.activation(
                    xn[:], x_t[:], AF.Identity, scale=rstd[:], bias=nmr[:],
                )
                nc.gpsimd.tensor_mul(xn[:], xn[:], gamma_sb[:])
                nc.gpsimd.tensor_add(xn[:], xn[:], beta_sb[:])
                nc.gpsimd.dma_start(y16[r0:r0 + 128, :, :], xn[:])

            with nc.named_scope("outproj"):
                for c in range(NSC):
                    for mq in range(4):
                        r0 = c * SCW + mq * 128
                        rt = lpool.tile([128, OC4, 512], F16, tag="resid",
                                        bufs=2, name=f"res{c}{mq}")
                        nc.scalar.dma_start(rt[:], resid16[r0:r0 + 128, :, :])
                        x_t = lpool.tile([128, OC4, 512], F16, tag="x", bufs=2,
                                         name=f"x{c}{mq}")
                        for n in range(OC4):
                            po = mpsum.tile([128, 512], F32, tag="po", bufs=2)
                            for cc in range(NH):
                                nc.tensor.matmul(
                                    po[:],
                                    ct_t[c][:, cc, mq * 128:(mq + 1) * 128],
                                    wo_sb[:, cc, n * 512:(n + 1) * 512],
                                    start=(cc == 0), stop=(cc == NH - 1),
                                )
                            nc.vector.tensor_tensor(
                                x_t[:, n, :], po[:], rt[:, n, :], ALU.add,
                            )
                        with nc.named_scope("ln"):
                            ln_tile(c, mq, x_t)

    nc.compile()
    return nc


def _get_nc():
    if "nc" not in _CACHE:
        _CACHE["nc"] = _build()
    return _CACHE["nc"]


def _prep_in_maps(hidden_states, audio_tokens, attention_mask,
                  Wq, bq, Wk, bk, Wv, bv, Wo, bo, gamma, beta):
    f = np.float32
    h16 = np.float16
    hs = np.asarray(hidden_states, f)
    au = np.asarray(audio_tokens, f)
    am = np.asarray(attention_mask, f)
    Wq, bq = np.asarray(Wq, f), np.asarray(bq, f)
    Wk, bk = np.asarray(Wk, f), np.asarray(bk, f)
    Wv, bv = np.asarray(Wv, f), np.asarray(bv, f)
    Wo, bo = np.asarray(Wo, f), np.asarray(bo, f)
    gamma, beta = np.asarray(gamma, f), np.asarray(beta, f)

    bo_eff = bo + bv @ Wo  # fold the V bias through the output projection
    ones = np.ones((128, 128), h16)
    gamma_b = np.ascontiguousarray(
        np.broadcast_to(gamma, (128, H))).astype(h16).reshape(128, OC4, 512)
    beta_b = np.ascontiguousarray(
        np.broadcast_to(beta, (128, H))).astype(h16).reshape(128, OC4, 512)

    in_maps = []
    for b in range(B):
        autb = np.ascontiguousarray(
            au[b].T.reshape(HK, 128, A).transpose(1, 0, 2)).astype(h16)
        maskT = np.ascontiguousarray((am[b] * -10000.0).reshape(AC, 128).T)
        for g in range(G):
            own = slice(g * HG, (g + 1) * HG)
            # head-slot order: own heads first, then peer heads
            order = list(range(g * NHG, (g + 1) * NHG)) + \
                list(range((1 - g) * NHG, (2 - g) * NHG))
            rows = slice(g * SL, (g + 1) * SL)
            Wq_p = Wq.reshape(H, NH, DH)[:, order, :].reshape(H, H)
            Wo_p = Wo.reshape(NH, DH, H)[order].reshape(H, H)
            bq_p = bq.reshape(NH, DH)[order]
            in_maps.append({
                "xt16": np.ascontiguousarray(
                    hs[b][rows].T.reshape(HK, 128, SL).transpose(1, 0, 2)
                ).astype(h16),
                "aut16": autb,
                "wq16": np.ascontiguousarray(
                    Wq_p.reshape(HK, 128, H).transpose(1, 0, 2)
                ).astype(h16),
                "wk16": np.ascontiguousarray(
                    Wk[:, own].reshape(HK, 128, HG).transpose(1, 0, 2)
                ).astype(h16),
                "wv16": np.ascontiguousarray(
                    Wv[:, own].reshape(HK, 128, HG).transpose(1, 0, 2)
                ).astype(h16),
                "wo16": np.ascontiguousarray(
                    Wo_p.reshape(NH, 128, H).transpose(1, 0, 2)
                ).astype(h16),
                "ones16": ones,
                "bqT": np.ascontiguousarray(bq_p.reshape(NH, 128).T),
                "bkT": np.ascontiguousarray(bk[own].reshape(NHG, 128).T),
                "maskT": maskT,
                "resid16": (hs[b][rows] + bo_eff[None, :]).astype(h16)
                .reshape(SL, OC4, 512),
                "gamma16": gamma_b,
                "beta16": beta_b,
            })
    return in_maps


def run_sharded(in_maps, trace=False):
    from concourse.bass_utils import run_bass_kernel_spmd

    nc = _get_nc()
    return run_bass_kernel_spmd(
        nc, in_maps, core_ids=list(range(8)), trace=trace,
        trace_cores=[0] if trace else None,
    )


def kernel(**inputs) -> np.ndarray:
    in_maps = _prep_in_maps(**inputs)
    trace = bool(int(os.environ.get("BASS_KERNEL_TRACE", "0")))
    r = run_sharded(in_maps, trace=trace)
    _CACHE["last_result"] = r
    out = np.empty((B, S, H), np.float32)
    for b in range(B):
        for g in range(G):
            out[b][g * SL:(g + 1) * SL] = (
                r.results[b * G + g]["y16"].astype(np.float32)
                .reshape(SL, H)
            )
    return out
